# revision 47
# baseline (speedup 1.0000x reference)
"""DeepSeekMoE block on 8 Trainium2 NeuronCores (expert-parallel).

Math (see the module's reference):
    xn    = rmsnorm(x) * norm_w
    shared = sum_e silu(xn @ Ws1[e] + bs1[e]) @ Ws2[e] + bs2[e]
    aff   = xn @ Wr.T ; top2 experts per token, gate = raw affinity
    routed = sum_{e in top2} aff_e * (silu(xn @ W1[e] + b1[e]) @ W2[e] + b2[e])
    out   = x + shared + routed

Distribution (SPMD, one program on all 8 cores; per-core data differs):
    * Routed experts: core e owns expert e's weights.  The host computes the
      top-2 routing (selection only) and gathers each expert's tokens into a
      fixed-capacity buffer; the device recomputes the norm and the gate
      affinity values in-kernel and produces gated expert outputs, which the
      host scatter-adds back (the host gather/scatter plays the role of the
      all-to-all dispatch/combine collectives).
    * Shared experts: core e owns shared expert e//4 over token quarter e%4
      (512 tokens) -- 8 equal slices of the 2*2048 token-passes.
    * Host sums the partials (allreduce-equivalent) and adds the residual x.

Device layout: activations kept transposed [d, token] so weights are the
stationary matmul operand (float32r for full-rate fp32 streaming).  Weights
are passed output-major ([HB, D, 128] / [DB, H, 128]) so each arriving tile
closes a complete PSUM accumulation group and the PE streams at DMA pace.
Per-token scalars (1/rms, gate) live in [1, T] rows broadcast across
partitions via a K=1 ones-matmul.
"""

import numpy as np

D, H, NT = 1024, 1408, 2048
TS = 512            # shared-expert token slice per core
C = 544             # routed-expert token capacity per core (max load is 542;
                    # _run rebuilds wider if routing ever overflows)
DB, HB = D // 128, H // 128
EPS = 1.1920929e-07  # float32 finfo eps, matches torch RMSNorm eps=None

MM_DT = "float32r"   # dtype for the large FFN matmuls
AUX_DT = "float32r"  # dtype for ssq/affinity matmuls
SILU_DECOMPOSE = False  # True: sigmoid+mul instead of the Silu LUT

_CACHE = {}


def _build(mm_dt_name=MM_DT, aux_dt_name=AUX_DT, ts=TS, c=C,
           silu_decompose=False):
    import concourse.bass as bass
    import concourse.bacc as bacc
    import concourse.mybir as mybir
    import concourse.tile as tile
    from contextlib import ExitStack

    f32 = mybir.dt.float32
    AF = mybir.ActivationFunctionType
    mm_dt = getattr(mybir.dt, mm_dt_name)
    aux_dt = getattr(mybir.dt, aux_dt_name)

    nc = bacc.Bacc("TRN2", target_bir_lowering=False, debug=False)
    xsT_d = nc.declare_dram_parameter("xsT", [D, ts], mm_dt, isOutput=False)
    xgT_d = nc.declare_dram_parameter("xgT", [D, c], mm_dt, isOutput=False)
    w1s_d = nc.declare_dram_parameter("w1s", [HB, D, 128], mm_dt, isOutput=False)
    w2s_d = nc.declare_dram_parameter("w2s", [DB, H, 128], mm_dt, isOutput=False)
    w1r_d = nc.declare_dram_parameter("w1r", [HB, D, 128], mm_dt, isOutput=False)
    w2r_d = nc.declare_dram_parameter("w2r", [DB, H, 128], mm_dt, isOutput=False)
    bias_d = nc.declare_dram_parameter("biases", [128, 2 * (HB + DB)], f32,
                                       isOutput=False)
    wro_d = nc.declare_dram_parameter("wro", [128, DB + 1], mm_dt,
                                      isOutput=False)
    onesrow_d = nc.declare_dram_parameter("onesrow", [1, 128], mm_dt,
                                          isOutput=False)
    osh_d = nc.declare_dram_parameter("out_sh", [D, ts], f32, isOutput=True)
    ort_d = nc.declare_dram_parameter("out_rt", [D, c], f32, isOutput=True)

    with ExitStack() as ctx:
        tc = ctx.enter_context(tile.TileContext(nc))
        consts = ctx.enter_context(tc.tile_pool(name="consts", bufs=1))
        xg_pool = ctx.enter_context(tc.tile_pool(name="xg", bufs=8))
        xs_pool = ctx.enter_context(tc.tile_pool(name="xs", bufs=8))
        sq_pool = ctx.enter_context(tc.tile_pool(name="sq", bufs=3))
        hg_pool = ctx.enter_context(tc.tile_pool(name="hg", bufs=24))
        hs_pool = ctx.enter_context(tc.tile_pool(name="hs", bufs=12))
        w1_pool = ctx.enter_context(tc.tile_pool(name="w1", bufs=14))
        w2_pool = ctx.enter_context(tc.tile_pool(name="w2", bufs=6))
        out_pool = ctx.enter_context(tc.tile_pool(name="outsb", bufs=3))
        bc_pool = ctx.enter_context(tc.tile_pool(name="bcast", bufs=3))
        row_pool = ctx.enter_context(tc.tile_pool(name="rows", bufs=3))
        ps_h = ctx.enter_context(tc.tile_pool(name="ps_h", bufs=4, space="PSUM"))
        ps_o = ctx.enter_context(tc.tile_pool(name="ps_o", bufs=2, space="PSUM"))
        ps_m = ctx.enter_context(tc.tile_pool(name="ps_m", bufs=1, space="PSUM"))

        wro_sb = consts.tile([128, DB + 1], mm_dt, tag="wro")
        nc.gpsimd.dma_start(out=wro_sb, in_=wro_d[:, :])
        wre_sb = wro_sb[:, 0:DB]
        ones_col = wro_sb[:, DB:DB + 1]
        eps_row = consts.tile([1, 1], f32)
        nc.vector.memset(eps_row, EPS)
        ones_row = consts.tile([1, 128], mm_dt, tag="onesrow")
        nc.gpsimd.dma_start(out=ones_row, in_=onesrow_d[:, :])

        def bcast(dst, row, tn):
            # broadcast a [1, tn] row across 128 partitions via a K=1 matmul
            row_r = row_pool.tile([1, tn], mm_dt, tag="rowr", bufs=2)
            nc.scalar.activation(row_r, row, AF.Copy)
            ps = ps_m.tile([128, tn], f32, tag="psbc")
            nc.tensor.matmul(ps, ones_row, row_r, start=True, stop=True)
            nc.scalar.activation(dst, ps, AF.Copy)

        # biases, resident for the whole kernel (one merged DMA)
        bias_sb = consts.tile([128, 2 * (HB + DB)], f32, tag="biases")
        nc.gpsimd.dma_start(out=bias_sb, in_=bias_d[:, :])
        b1s_sb = bias_sb[:, 0:HB]
        b2s_sb = bias_sb[:, HB:HB + DB]
        b1r_sb = bias_sb[:, HB + DB:2 * HB + DB]
        b2r_sb = bias_sb[:, 2 * HB + DB:2 * (HB + DB)]

        def ffn_job(xT_dram, T, tslices, w1_dram, w2_dram, b1_sb, b2_sb,
                    out_dram, routed, xpool, hpool, xtag):
            # ---- load activations (transposed [d, token]), slice-major ----
            xt = [xpool.tile([128, T], mm_dt, tag=xtag, name=f"{xtag}{dk}")
                  for dk in range(DB)]
            w1_tiles = {}

            def w1_get(hb):
                if hb not in w1_tiles:
                    t = w1_pool.tile([128, DB, 128], mm_dt, tag="w1")
                    nc.sync.dma_start(
                        out=t,
                        in_=w1_dram[hb].rearrange("(a p) w -> p a w", p=128))
                    w1_tiles[hb] = t
                return w1_tiles[hb]

            for i, (t0, tn) in enumerate(tslices):
                for dk in range(DB):
                    nc.sync.dma_start(
                        out=xt[dk][:, t0:t0 + tn],
                        in_=xT_dram[dk * 128:(dk + 1) * 128, t0:t0 + tn])
                if i == 0:
                    w1_get(0)
                    w1_get(1)

            rs_b = bc_pool.tile([128, T], f32, tag="bc")
            gate_b = None
            if routed:
                gate_b = bc_pool.tile([128, T], f32, tag="bc")

            for (t0, tn) in tslices:
                sl = slice(t0, t0 + tn)
                # ---- 1/rms row:  rs = 1/sqrt(mean(x^2) + eps) ----
                ps_ssq = ps_m.tile([1, tn], f32, tag="psrow")
                for dk in range(DB):
                    sq = sq_pool.tile([128, tn], mm_dt, tag="sqr")
                    nc.scalar.activation(sq, xt[dk][:, sl], AF.Square)
                    nc.tensor.matmul(ps_ssq, ones_col, sq,
                                     start=(dk == 0), stop=(dk == DB - 1))
                r_sqrt = row_pool.tile([1, tn], f32, tag="row")
                nc.scalar.activation(r_sqrt, ps_ssq, AF.Sqrt,
                                     bias=eps_row, scale=1.0 / D)
                r_rs = row_pool.tile([1, tn], f32, tag="row")
                nc.vector.reciprocal(r_rs, r_sqrt)
                bcast(rs_b[:, sl], r_rs, tn)

                if routed:
                    # gate row = affinity = (x @ wre) * rs  (wre norm_w-folded)
                    ps_aff = ps_m.tile([1, tn], f32, tag="psrow")
                    for dk in range(DB):
                        nc.tensor.matmul(ps_aff, wre_sb[:, dk:dk + 1],
                                         xt[dk][:, sl],
                                         start=(dk == 0), stop=(dk == DB - 1))
                    r_gate = row_pool.tile([1, tn], f32, tag="row")
                    nc.vector.tensor_mul(r_gate, ps_aff, r_rs)
                    bcast(gate_b[:, sl], r_gate, tn)

            # ---- mm1 + silu, hb-major: one weight tile closes a full group
            ht = {}
            for si, (t0, tn) in enumerate(tslices):
                for hb in range(HB):
                    w1t = w1_get(hb)
                    sl = slice(t0, t0 + tn)
                    psh = ps_h.tile([128, tn], f32, tag="psh")
                    for dk in range(DB):
                        nc.tensor.matmul(psh, w1t[:, dk, :], xt[dk][:, sl],
                                         start=(dk == 0), stop=(dk == DB - 1))
                    tmp = sq_pool.tile([128, tn], f32, tag="sq")
                    nc.vector.tensor_mul(tmp, psh, rs_b[:, sl])
                    h = hpool.tile([128, tn], mm_dt, tag="h")
                    if silu_decompose:
                        sg = sq_pool.tile([128, tn], f32, tag="sq")
                        nc.scalar.activation(sg, tmp, AF.Sigmoid,
                                             bias=b1_sb[:, hb:hb + 1])
                        nc.vector.scalar_tensor_tensor(
                            h, tmp, b1_sb[:, hb:hb + 1], sg,
                            op0=mybir.AluOpType.add,
                            op1=mybir.AluOpType.mult)
                    else:
                        nc.scalar.activation(h, tmp, AF.Silu,
                                             bias=b1_sb[:, hb:hb + 1])
                    ht[(hb, si)] = h

            # ---- mm2 + combine, db-major ----
            for db in range(DB):
                w2t = w2_pool.tile([128, HB, 128], mm_dt, tag="w2")
                nc.sync.dma_start(
                    out=w2t,
                    in_=w2_dram[db].rearrange("(a p) w -> p a w", p=128))
                for si, (t0, tn) in enumerate(tslices):
                    sl = slice(t0, t0 + tn)
                    pso = ps_o.tile([128, tn], f32, tag="pso")
                    for hb in range(HB):
                        nc.tensor.matmul(pso, w2t[:, hb, :], ht[(hb, si)],
                                         start=(hb == 0), stop=(hb == HB - 1))
                    osb = out_pool.tile([128, tn], f32, tag="osb")
                    if routed:
                        nc.vector.scalar_tensor_tensor(
                            osb, pso, b2_sb[:, db:db + 1], gate_b[:, sl],
                            op0=mybir.AluOpType.add,
                            op1=mybir.AluOpType.mult)
                    else:
                        nc.scalar.activation(osb, pso, AF.Identity,
                                             bias=b2_sb[:, db:db + 1])
                    nc.sync.dma_start(
                        out=out_dram[db * 128:(db + 1) * 128, sl], in_=osb)

        # routed job (bigger) first so its weight DMAs start immediately
        nsl = max(1, -(-c // 512))
        base, rem = divmod(c, nsl)
        tsl2, off = [], 0
        for i in range(nsl):
            tn = base + (1 if i < rem else 0)
            tsl2.append((off, tn))
            off += tn
        ffn_job(xgT_d, c, tsl2,
                w1r_d, w2r_d, b1r_sb, b2r_sb, ort_d, True,
                xg_pool, hg_pool, "xg")
        ffn_job(xsT_d, ts, [(0, ts)], w1s_d, w2s_d,
                b1s_sb, b2s_sb, osh_d, False, xs_pool, hs_pool, "xs")

    nc.compile()
    return nc


def _get_nc(key):
    if key not in _CACHE:
        _CACHE[key] = _build(*key)
    return _CACHE[key]


def kernel(x, norm_w, Wr, Ws1, bs1, Ws2, bs2, W1, b1, W2, b2):
    out, _ = _run(x, norm_w, Wr, Ws1, bs1, Ws2, bs2, W1, b1, W2, b2)
    return out


def _run(x, norm_w, Wr, Ws1, bs1, Ws2, bs2, W1, b1, W2, b2,
         trace=False, trace_kwargs=None):
    from concourse.bass_utils import run_bass_kernel_spmd

    x = np.ascontiguousarray(np.asarray(x, dtype=np.float32))
    B, S, _ = x.shape
    xf = x.reshape(NT, D)
    norm_w = np.asarray(norm_w, dtype=np.float32)
    nw64 = norm_w.astype(np.float64)

    # ---- host: routing decisions only (selection, not values) ----
    ms = np.mean(xf.astype(np.float64) ** 2, axis=-1, keepdims=True)
    xn64 = xf.astype(np.float64) / np.sqrt(ms + EPS) * nw64
    aff = xn64 @ np.asarray(Wr, dtype=np.float32).astype(np.float64).T
    part = np.argpartition(-aff, 2, axis=1)[:, :2]  # top-2 set per token
    onehot = np.zeros((NT, 8), dtype=bool)
    onehot[np.arange(NT)[:, None], part] = True
    sel = [np.nonzero(onehot[:, e])[0] for e in range(8)]
    counts = np.array([len(s) for s in sel])
    c_eff = C
    if counts.max() > C:  # input drift beyond planned capacity: rebuild wider
        c_eff = int(-(-int(counts.max()) // 128) * 128)

    # fold norm_w into the first-layer weights (exact when norm_w == 1)
    W1f = (np.asarray(W1, np.float64) * nw64[None, :, None]).astype(np.float32)
    Ws1f = (np.asarray(Ws1, np.float64) * nw64[None, :, None]).astype(np.float32)
    Wrf = (np.asarray(Wr, np.float64) * nw64[None, :]).astype(np.float32)
    Ws2 = np.asarray(Ws2, np.float32)
    W2 = np.asarray(W2, np.float32)
    bs1 = np.asarray(bs1, np.float32)
    bs2 = np.asarray(bs2, np.float32)
    b1 = np.asarray(b1, np.float32)
    b2 = np.asarray(b2, np.float32)

    in_maps = []
    for e in range(8):
        se, q = divmod(e, 4)
        tsl = slice(q * TS, (q + 1) * TS)
        xg = np.zeros((c_eff, D), np.float32)
        xg[:counts[e]] = xf[sel[e]]
        in_maps.append(dict(
            xsT=np.ascontiguousarray(xf[tsl].T),
            xgT=np.ascontiguousarray(xg.T),
            w1s=np.ascontiguousarray(
                Ws1f[se].reshape(D, HB, 128).transpose(1, 0, 2)),
            w2s=np.ascontiguousarray(
                Ws2[se].reshape(H, DB, 128).transpose(1, 0, 2)),
            w1r=np.ascontiguousarray(
                W1f[e].reshape(D, HB, 128).transpose(1, 0, 2)),
            w2r=np.ascontiguousarray(
                W2[e].reshape(H, DB, 128).transpose(1, 0, 2)),
            biases=np.ascontiguousarray(np.concatenate([
                bs1[se, 0].reshape(HB, 128).T,
                bs2[se, 0].reshape(DB, 128).T,
                b1[e, 0].reshape(HB, 128).T,
                b2[e, 0].reshape(DB, 128).T], axis=1)),
            wro=np.ascontiguousarray(np.concatenate([
                Wrf[e].reshape(DB, 128).T,
                np.ones((128, 1), np.float32)], axis=1)),
            onesrow=np.ones((1, 128), np.float32),
        ))

    nc = _get_nc((MM_DT, AUX_DT, TS, c_eff, SILU_DECOMPOSE))
    kw = dict(trace_kwargs or {})
    res = run_bass_kernel_spmd(nc, in_maps, list(range(8)), trace=trace, **kw)

    # ---- host: combine partials (allreduce / all-to-all-combine stand-in) ----
    outf = xf.astype(np.float32).copy()
    for e in range(8):
        q = e % 4
        outf[q * TS:(q + 1) * TS] += res.results[e]["out_sh"].T
        outf[sel[e]] += res.results[e]["out_rt"][:, :counts[e]].T
    return outf.reshape(B, S, D), res


# revision 53
# speedup vs baseline: 1.0057x; 1.0057x over previous
"""DeepSeekMoE block on 8 Trainium2 NeuronCores (expert-parallel).

Math (see the module's reference):
    xn    = rmsnorm(x) * norm_w
    shared = sum_e silu(xn @ Ws1[e] + bs1[e]) @ Ws2[e] + bs2[e]
    aff   = xn @ Wr.T ; top2 experts per token, gate = raw affinity
    routed = sum_{e in top2} aff_e * (silu(xn @ W1[e] + b1[e]) @ W2[e] + b2[e])
    out   = x + shared + routed

Distribution (SPMD, one program on all 8 cores; per-core data differs):
    * Routed experts: core e owns expert e's weights.  The host computes the
      top-2 routing (selection only) and gathers each expert's tokens into a
      fixed-capacity buffer; the device recomputes the norm and the gate
      affinity values in-kernel and produces gated expert outputs, which the
      host scatter-adds back (the host gather/scatter plays the role of the
      all-to-all dispatch/combine collectives).
    * Shared experts: core e owns shared expert e//4 over token quarter e%4
      (512 tokens) -- 8 equal slices of the 2*2048 token-passes.
    * Host sums the partials (allreduce-equivalent) and adds the residual x.

Device layout: activations kept transposed [d, token] so weights are the
stationary matmul operand (float32r for full-rate fp32 streaming).  Weights
are passed output-major ([HB, D, 128] / [DB, H, 128]) so each arriving tile
closes a complete PSUM accumulation group and the PE streams at DMA pace.
Per-token scalars (1/rms, gate) live in [1, T] rows broadcast across
partitions via a K=1 ones-matmul.
"""

import numpy as np

D, H, NT = 1024, 1408, 2048
TS = 512            # shared-expert token slice per core
C = 544             # routed-expert token capacity per core (max load is 542;
                    # _run rebuilds wider if routing ever overflows)
DB, HB = D // 128, H // 128
EPS = 1.1920929e-07  # float32 finfo eps, matches torch RMSNorm eps=None

MM_DT = "float32r"   # dtype for the large FFN matmuls
AUX_DT = "float32r"  # dtype for ssq/affinity matmuls
SILU_DECOMPOSE = False  # True: sigmoid+mul instead of the Silu LUT

_CACHE = {}


def _build(mm_dt_name=MM_DT, aux_dt_name=AUX_DT, ts=TS, c=C,
           silu_decompose=False):
    import concourse.bass as bass
    import concourse.bacc as bacc
    import concourse.mybir as mybir
    import concourse.tile as tile
    from contextlib import ExitStack

    f32 = mybir.dt.float32
    AF = mybir.ActivationFunctionType
    mm_dt = getattr(mybir.dt, mm_dt_name)
    aux_dt = getattr(mybir.dt, aux_dt_name)

    nc = bacc.Bacc("TRN2", target_bir_lowering=False, debug=False)
    xsT_d = nc.declare_dram_parameter("xsT", [D, ts], mm_dt, isOutput=False)
    xgT_d = nc.declare_dram_parameter("xgT", [D, c], mm_dt, isOutput=False)
    w1s_d = nc.declare_dram_parameter("w1s", [HB, D, 128], mm_dt, isOutput=False)
    w2s_d = nc.declare_dram_parameter("w2s", [DB, H, 128], mm_dt, isOutput=False)
    w1r_d = nc.declare_dram_parameter("w1r", [HB, D, 128], mm_dt, isOutput=False)
    w2r_d = nc.declare_dram_parameter("w2r", [DB, H, 128], mm_dt, isOutput=False)
    bias_d = nc.declare_dram_parameter("biases", [128, 2 * (HB + DB)], f32,
                                       isOutput=False)
    wro_d = nc.declare_dram_parameter("wro", [128, DB + 1], mm_dt,
                                      isOutput=False)
    onesrow_d = nc.declare_dram_parameter("onesrow", [1, 128], mm_dt,
                                          isOutput=False)
    osh_d = nc.declare_dram_parameter("out_sh", [D, ts], f32, isOutput=True)
    ort_d = nc.declare_dram_parameter("out_rt", [D, c], f32, isOutput=True)

    with ExitStack() as ctx:
        tc = ctx.enter_context(tile.TileContext(nc))
        consts = ctx.enter_context(tc.tile_pool(name="consts", bufs=1))
        xg_pool = ctx.enter_context(tc.tile_pool(name="xg", bufs=8))
        xs_pool = ctx.enter_context(tc.tile_pool(name="xs", bufs=8))
        sq_pool = ctx.enter_context(tc.tile_pool(name="sq", bufs=3))
        hg_pool = ctx.enter_context(tc.tile_pool(name="hg", bufs=24))
        hs_pool = ctx.enter_context(tc.tile_pool(name="hs", bufs=12))
        w1_pool = ctx.enter_context(tc.tile_pool(name="w1", bufs=14))
        w2_pool = ctx.enter_context(tc.tile_pool(name="w2", bufs=6))
        out_pool = ctx.enter_context(tc.tile_pool(name="outsb", bufs=3))
        bc_pool = ctx.enter_context(tc.tile_pool(name="bcast", bufs=3))
        row_pool = ctx.enter_context(tc.tile_pool(name="rows", bufs=3))
        ps_h = ctx.enter_context(tc.tile_pool(name="ps_h", bufs=4, space="PSUM"))
        ps_o = ctx.enter_context(tc.tile_pool(name="ps_o", bufs=2, space="PSUM"))
        ps_m = ctx.enter_context(tc.tile_pool(name="ps_m", bufs=1, space="PSUM"))

        wro_sb = consts.tile([128, DB + 1], mm_dt, tag="wro")
        nc.gpsimd.dma_start(out=wro_sb, in_=wro_d[:, :])
        wre_sb = wro_sb[:, 0:DB]
        ones_col = wro_sb[:, DB:DB + 1]
        eps_row = consts.tile([1, 1], f32)
        nc.vector.memset(eps_row, EPS)
        ones_row = consts.tile([1, 128], mm_dt, tag="onesrow")
        nc.gpsimd.dma_start(out=ones_row, in_=onesrow_d[:, :])

        def bcast(dst, row, tn):
            # broadcast a [1, tn] row across 128 partitions via a K=1 matmul
            row_r = row_pool.tile([1, tn], mm_dt, tag="rowr", bufs=2)
            nc.scalar.activation(row_r, row, AF.Copy)
            ps = ps_m.tile([128, tn], f32, tag="psbc")
            nc.tensor.matmul(ps, ones_row, row_r, start=True, stop=True)
            nc.scalar.activation(dst, ps, AF.Copy)

        # PE warm-up: keep TensorE continuously busy through the DMA-bound
        # prefix so the HAM clock gate reaches 2.4GHz before the real
        # matmul stream begins (output is never read).
        ps_warm = ps_m.tile([128, 128], f32, tag="psbc")
        for k in range(8):
            nc.tensor.matmul(ps_warm, ones_row, ones_row,
                             start=(k == 0), stop=(k == 7))
        warm_sink = row_pool.tile([1, 128], f32, tag="row")
        nc.scalar.activation(warm_sink, ps_warm[0:1, :], AF.Copy)

        # biases, resident for the whole kernel (one merged DMA)
        bias_sb = consts.tile([128, 2 * (HB + DB)], f32, tag="biases")
        nc.gpsimd.dma_start(out=bias_sb, in_=bias_d[:, :])
        b1s_sb = bias_sb[:, 0:HB]
        b2s_sb = bias_sb[:, HB:HB + DB]
        b1r_sb = bias_sb[:, HB + DB:2 * HB + DB]
        b2r_sb = bias_sb[:, 2 * HB + DB:2 * (HB + DB)]

        def ffn_job(xT_dram, T, tslices, w1_dram, w2_dram, b1_sb, b2_sb,
                    out_dram, routed, xpool, hpool, xtag, oslices=None):
            oslices = oslices or tslices
            tn_h = tslices[0][1]
            # ---- load activations (transposed [d, token]), slice-major ----
            xt = [xpool.tile([128, T], mm_dt, tag=xtag, name=f"{xtag}{dk}")
                  for dk in range(DB)]
            w1_tiles = {}

            def w1_get(hb):
                if hb not in w1_tiles:
                    t = w1_pool.tile([128, DB, 128], mm_dt, tag="w1")
                    nc.sync.dma_start(
                        out=t,
                        in_=w1_dram[hb].rearrange("(a p) w -> p a w", p=128))
                    w1_tiles[hb] = t
                return w1_tiles[hb]

            for i, (t0, tn) in enumerate(tslices):
                for dk in range(DB):
                    nc.sync.dma_start(
                        out=xt[dk][:, t0:t0 + tn],
                        in_=xT_dram[dk * 128:(dk + 1) * 128, t0:t0 + tn])
                if i == 0:
                    w1_get(0)
                    w1_get(1)

            rs_b = bc_pool.tile([128, T], f32, tag="bc")
            gate_b = None
            if routed:
                gate_b = bc_pool.tile([128, T], f32, tag="bc")

            for (t0, tn) in tslices:
                sl = slice(t0, t0 + tn)
                # ---- 1/rms row:  rs = 1/sqrt(mean(x^2) + eps) ----
                ps_ssq = ps_m.tile([1, tn], f32, tag="psrow")
                for dk in range(DB):
                    sq = sq_pool.tile([128, tn], mm_dt, tag="sqr")
                    nc.scalar.activation(sq, xt[dk][:, sl], AF.Square)
                    nc.tensor.matmul(ps_ssq, ones_col, sq,
                                     start=(dk == 0), stop=(dk == DB - 1))
                r_sqrt = row_pool.tile([1, tn], f32, tag="row")
                nc.scalar.activation(r_sqrt, ps_ssq, AF.Sqrt,
                                     bias=eps_row, scale=1.0 / D)
                r_rs = row_pool.tile([1, tn], f32, tag="row")
                nc.vector.reciprocal(r_rs, r_sqrt)
                bcast(rs_b[:, sl], r_rs, tn)

                if routed:
                    # gate row = affinity = (x @ wre) * rs  (wre norm_w-folded)
                    ps_aff = ps_m.tile([1, tn], f32, tag="psrow")
                    for dk in range(DB):
                        nc.tensor.matmul(ps_aff, wre_sb[:, dk:dk + 1],
                                         xt[dk][:, sl],
                                         start=(dk == 0), stop=(dk == DB - 1))
                    r_gate = row_pool.tile([1, tn], f32, tag="row")
                    nc.vector.tensor_mul(r_gate, ps_aff, r_rs)
                    bcast(gate_b[:, sl], r_gate, tn)

            # ---- mm1 + silu, hb-major: one weight tile closes a full group
            ht = {}
            for si, (t0, tn) in enumerate(tslices):
                for hb in range(HB):
                    w1t = w1_get(hb)
                    sl = slice(t0, t0 + tn)
                    psh = ps_h.tile([128, tn], f32, tag="psh")
                    for dk in range(DB):
                        nc.tensor.matmul(psh, w1t[:, dk, :], xt[dk][:, sl],
                                         start=(dk == 0), stop=(dk == DB - 1))
                    tmp = sq_pool.tile([128, tn], f32, tag="sq")
                    nc.vector.tensor_mul(tmp, psh, rs_b[:, sl])
                    h = hpool.tile([128, tn], mm_dt, tag="h")
                    if silu_decompose:
                        sg = sq_pool.tile([128, tn], f32, tag="sq")
                        nc.scalar.activation(sg, tmp, AF.Sigmoid,
                                             bias=b1_sb[:, hb:hb + 1])
                        nc.vector.scalar_tensor_tensor(
                            h, tmp, b1_sb[:, hb:hb + 1], sg,
                            op0=mybir.AluOpType.add,
                            op1=mybir.AluOpType.mult)
                    else:
                        nc.scalar.activation(h, tmp, AF.Silu,
                                             bias=b1_sb[:, hb:hb + 1])
                    ht[(hb, si)] = h

            # ---- mm2 + combine, db-major ----
            for db in range(DB):
                w2t = w2_pool.tile([128, HB, 128], mm_dt, tag="w2")
                nc.sync.dma_start(
                    out=w2t,
                    in_=w2_dram[db].rearrange("(a p) w -> p a w", p=128))
                for (t0, tn) in (oslices if db >= DB - 2 else tslices):
                    sl = slice(t0, t0 + tn)
                    hi, ho = t0 // tn_h, t0 % tn_h
                    pso = ps_o.tile([128, tn], f32, tag="pso")
                    for hb in range(HB):
                        nc.tensor.matmul(pso, w2t[:, hb, :],
                                         ht[(hb, hi)][:, ho:ho + tn],
                                         start=(hb == 0), stop=(hb == HB - 1))
                    osb = out_pool.tile([128, tn], f32, tag="osb")
                    if routed:
                        nc.vector.scalar_tensor_tensor(
                            osb, pso, b2_sb[:, db:db + 1], gate_b[:, sl],
                            op0=mybir.AluOpType.add,
                            op1=mybir.AluOpType.mult)
                    else:
                        nc.scalar.activation(osb, pso, AF.Identity,
                                             bias=b2_sb[:, db:db + 1])
                    nc.sync.dma_start(
                        out=out_dram[db * 128:(db + 1) * 128, sl], in_=osb)

        # routed job (bigger) first so its weight DMAs start immediately
        nsl = max(1, -(-c // 512))
        base, rem = divmod(c, nsl)
        tsl2, off = [], 0
        for i in range(nsl):
            tn = base + (1 if i < rem else 0)
            tsl2.append((off, tn))
            off += tn
        ffn_job(xgT_d, c, tsl2,
                w1r_d, w2r_d, b1r_sb, b2r_sb, ort_d, True,
                xg_pool, hg_pool, "xg")
        ffn_job(xsT_d, ts, [(0, ts)], w1s_d, w2s_d,
                b1s_sb, b2s_sb, osh_d, False, xs_pool, hs_pool, "xs",
                oslices=[(0, ts // 2), (ts // 2, ts // 2)])

    nc.compile()
    return nc


def _get_nc(key):
    if key not in _CACHE:
        _CACHE[key] = _build(*key)
    return _CACHE[key]


def kernel(x, norm_w, Wr, Ws1, bs1, Ws2, bs2, W1, b1, W2, b2):
    out, _ = _run(x, norm_w, Wr, Ws1, bs1, Ws2, bs2, W1, b1, W2, b2)
    return out


def _run(x, norm_w, Wr, Ws1, bs1, Ws2, bs2, W1, b1, W2, b2,
         trace=False, trace_kwargs=None):
    from concourse.bass_utils import run_bass_kernel_spmd

    x = np.ascontiguousarray(np.asarray(x, dtype=np.float32))
    B, S, _ = x.shape
    xf = x.reshape(NT, D)
    norm_w = np.asarray(norm_w, dtype=np.float32)
    nw64 = norm_w.astype(np.float64)

    # ---- host: routing decisions only (selection, not values) ----
    ms = np.mean(xf.astype(np.float64) ** 2, axis=-1, keepdims=True)
    xn64 = xf.astype(np.float64) / np.sqrt(ms + EPS) * nw64
    aff = xn64 @ np.asarray(Wr, dtype=np.float32).astype(np.float64).T
    part = np.argpartition(-aff, 2, axis=1)[:, :2]  # top-2 set per token
    onehot = np.zeros((NT, 8), dtype=bool)
    onehot[np.arange(NT)[:, None], part] = True
    sel = [np.nonzero(onehot[:, e])[0] for e in range(8)]
    counts = np.array([len(s) for s in sel])
    c_eff = C
    if counts.max() > C:  # input drift beyond planned capacity: rebuild wider
        c_eff = int(-(-int(counts.max()) // 128) * 128)

    # fold norm_w into the first-layer weights (exact when norm_w == 1)
    W1f = (np.asarray(W1, np.float64) * nw64[None, :, None]).astype(np.float32)
    Ws1f = (np.asarray(Ws1, np.float64) * nw64[None, :, None]).astype(np.float32)
    Wrf = (np.asarray(Wr, np.float64) * nw64[None, :]).astype(np.float32)
    Ws2 = np.asarray(Ws2, np.float32)
    W2 = np.asarray(W2, np.float32)
    bs1 = np.asarray(bs1, np.float32)
    bs2 = np.asarray(bs2, np.float32)
    b1 = np.asarray(b1, np.float32)
    b2 = np.asarray(b2, np.float32)

    in_maps = []
    for e in range(8):
        se, q = divmod(e, 4)
        tsl = slice(q * TS, (q + 1) * TS)
        xg = np.zeros((c_eff, D), np.float32)
        xg[:counts[e]] = xf[sel[e]]
        in_maps.append(dict(
            xsT=np.ascontiguousarray(xf[tsl].T),
            xgT=np.ascontiguousarray(xg.T),
            w1s=np.ascontiguousarray(
                Ws1f[se].reshape(D, HB, 128).transpose(1, 0, 2)),
            w2s=np.ascontiguousarray(
                Ws2[se].reshape(H, DB, 128).transpose(1, 0, 2)),
            w1r=np.ascontiguousarray(
                W1f[e].reshape(D, HB, 128).transpose(1, 0, 2)),
            w2r=np.ascontiguousarray(
                W2[e].reshape(H, DB, 128).transpose(1, 0, 2)),
            biases=np.ascontiguousarray(np.concatenate([
                bs1[se, 0].reshape(HB, 128).T,
                bs2[se, 0].reshape(DB, 128).T,
                b1[e, 0].reshape(HB, 128).T,
                b2[e, 0].reshape(DB, 128).T], axis=1)),
            wro=np.ascontiguousarray(np.concatenate([
                Wrf[e].reshape(DB, 128).T,
                np.ones((128, 1), np.float32)], axis=1)),
            onesrow=np.ones((1, 128), np.float32),
        ))

    nc = _get_nc((MM_DT, AUX_DT, TS, c_eff, SILU_DECOMPOSE))
    kw = dict(trace_kwargs or {})
    res = run_bass_kernel_spmd(nc, in_maps, list(range(8)), trace=trace, **kw)

    # ---- host: combine partials (allreduce / all-to-all-combine stand-in) ----
    outf = xf.astype(np.float32).copy()
    for e in range(8):
        q = e % 4
        outf[q * TS:(q + 1) * TS] += res.results[e]["out_sh"].T
        outf[sel[e]] += res.results[e]["out_rt"][:, :counts[e]].T
    return outf.reshape(B, S, D), res


# revision 59
# speedup vs baseline: 1.0073x; 1.0016x over previous
"""DeepSeekMoE block on 8 Trainium2 NeuronCores (expert-parallel).

Math (see the module's reference):
    xn    = rmsnorm(x) * norm_w
    shared = sum_e silu(xn @ Ws1[e] + bs1[e]) @ Ws2[e] + bs2[e]
    aff   = xn @ Wr.T ; top2 experts per token, gate = raw affinity
    routed = sum_{e in top2} aff_e * (silu(xn @ W1[e] + b1[e]) @ W2[e] + b2[e])
    out   = x + shared + routed

Distribution (SPMD, one program on all 8 cores; per-core data differs):
    * Routed experts: core e owns expert e's weights.  The host computes the
      top-2 routing (selection only) and gathers each expert's tokens into a
      fixed-capacity buffer; the device recomputes the norm and the gate
      affinity values in-kernel and produces gated expert outputs, which the
      host scatter-adds back (the host gather/scatter plays the role of the
      all-to-all dispatch/combine collectives).
    * Shared experts: core e owns shared expert e//4 over token quarter e%4
      (512 tokens) -- 8 equal slices of the 2*2048 token-passes.
    * Host sums the partials (allreduce-equivalent) and adds the residual x.

Device layout: activations kept transposed [d, token] so weights are the
stationary matmul operand (float32r for full-rate fp32 streaming).  Weights
are passed output-major ([HB, D, 128] / [DB, H, 128]) so each arriving tile
closes a complete PSUM accumulation group and the PE streams at DMA pace.
Per-token scalars (1/rms, gate) live in [1, T] rows broadcast across
partitions via a K=1 ones-matmul.
"""

import numpy as np

D, H, NT = 1024, 1408, 2048
TS = 512            # shared-expert token slice per core
C = 544             # routed-expert token capacity per core (max load is 542;
                    # _run rebuilds wider if routing ever overflows)
DB, HB = D // 128, H // 128
EPS = 1.1920929e-07  # float32 finfo eps, matches torch RMSNorm eps=None

MM_DT = "float32r"   # dtype for the large FFN matmuls
AUX_DT = "float32r"  # dtype for ssq/affinity matmuls
SILU_DECOMPOSE = False  # True: sigmoid+mul instead of the Silu LUT

_CACHE = {}


def _build(mm_dt_name=MM_DT, aux_dt_name=AUX_DT, ts=TS, c=C,
           silu_decompose=False):
    import concourse.bass as bass
    import concourse.bacc as bacc
    import concourse.mybir as mybir
    import concourse.tile as tile
    from contextlib import ExitStack

    f32 = mybir.dt.float32
    AF = mybir.ActivationFunctionType
    mm_dt = getattr(mybir.dt, mm_dt_name)
    aux_dt = getattr(mybir.dt, aux_dt_name)

    nc = bacc.Bacc("TRN2", target_bir_lowering=False, debug=False)
    xsT_d = nc.declare_dram_parameter("xsT", [D, ts], mm_dt, isOutput=False)
    xgT_d = nc.declare_dram_parameter("xgT", [D, c], mm_dt, isOutput=False)
    w1s_d = nc.declare_dram_parameter("w1s", [HB, D, 128], mm_dt, isOutput=False)
    w2s_d = nc.declare_dram_parameter("w2s", [DB, H, 128], mm_dt, isOutput=False)
    w1r_d = nc.declare_dram_parameter("w1r", [HB, D, 128], mm_dt, isOutput=False)
    w2r_d = nc.declare_dram_parameter("w2r", [DB, H, 128], mm_dt, isOutput=False)
    bias_d = nc.declare_dram_parameter("biases", [128, 2 * (HB + DB)], f32,
                                       isOutput=False)
    wro_d = nc.declare_dram_parameter("wro", [128, DB + 1], mm_dt,
                                      isOutput=False)
    onesrow_d = nc.declare_dram_parameter("onesrow", [1, 128], mm_dt,
                                          isOutput=False)
    osh_d = nc.declare_dram_parameter("out_sh", [D, ts], f32, isOutput=True)
    ort_d = nc.declare_dram_parameter("out_rt", [D, c], f32, isOutput=True)

    with ExitStack() as ctx:
        tc = ctx.enter_context(tile.TileContext(nc))
        consts = ctx.enter_context(tc.tile_pool(name="consts", bufs=1))
        xg_pool = ctx.enter_context(tc.tile_pool(name="xg", bufs=8))
        xs_pool = ctx.enter_context(tc.tile_pool(name="xs", bufs=8))
        sq_pool = ctx.enter_context(tc.tile_pool(name="sq", bufs=3))
        hg_pool = ctx.enter_context(tc.tile_pool(name="hg", bufs=24))
        hs_pool = ctx.enter_context(tc.tile_pool(name="hs", bufs=13))
        w1_pool = ctx.enter_context(tc.tile_pool(name="w1", bufs=13))
        w2_pool = ctx.enter_context(tc.tile_pool(name="w2", bufs=6))
        out_pool = ctx.enter_context(tc.tile_pool(name="outsb", bufs=4))
        bc_pool = ctx.enter_context(tc.tile_pool(name="bcast", bufs=3))
        row_pool = ctx.enter_context(tc.tile_pool(name="rows", bufs=3))
        ps_h = ctx.enter_context(tc.tile_pool(name="ps_h", bufs=3, space="PSUM"))
        ps_o = ctx.enter_context(tc.tile_pool(name="ps_o", bufs=3, space="PSUM"))
        ps_m = ctx.enter_context(tc.tile_pool(name="ps_m", bufs=1, space="PSUM"))

        wro_sb = consts.tile([128, DB + 1], mm_dt, tag="wro")
        nc.gpsimd.dma_start(out=wro_sb, in_=wro_d[:, :])
        wre_sb = wro_sb[:, 0:DB]
        ones_col = wro_sb[:, DB:DB + 1]
        eps_row = consts.tile([1, 1], f32)
        nc.vector.memset(eps_row, EPS)
        ones_row = consts.tile([1, 128], mm_dt, tag="onesrow")
        nc.gpsimd.dma_start(out=ones_row, in_=onesrow_d[:, :])

        def bcast(dst, row, tn):
            # broadcast a [1, tn] row across 128 partitions via a K=1 matmul
            row_r = row_pool.tile([1, tn], mm_dt, tag="rowr", bufs=2)
            nc.scalar.activation(row_r, row, AF.Copy)
            ps = ps_m.tile([128, tn], f32, tag="psbc")
            nc.tensor.matmul(ps, ones_row, row_r, start=True, stop=True)
            nc.scalar.activation(dst, ps, AF.Copy)

        # PE warm-up: keep TensorE continuously busy through the DMA-bound
        # prefix so the HAM clock gate reaches 2.4GHz before the real
        # matmul stream begins (output is never read).
        ps_warm = ps_m.tile([128, 128], f32, tag="psbc")
        for k in range(8):
            nc.tensor.matmul(ps_warm, ones_row, ones_row,
                             start=(k == 0), stop=(k == 7))
        warm_sink = row_pool.tile([1, 128], f32, tag="row")
        nc.scalar.activation(warm_sink, ps_warm[0:1, :], AF.Copy)

        # biases, resident for the whole kernel (one merged DMA)
        bias_sb = consts.tile([128, 2 * (HB + DB)], f32, tag="biases")
        nc.gpsimd.dma_start(out=bias_sb, in_=bias_d[:, :])
        b1s_sb = bias_sb[:, 0:HB]
        b2s_sb = bias_sb[:, HB:HB + DB]
        b1r_sb = bias_sb[:, HB + DB:2 * HB + DB]
        b2r_sb = bias_sb[:, 2 * HB + DB:2 * (HB + DB)]

        def ffn_job(xT_dram, T, tslices, w1_dram, w2_dram, b1_sb, b2_sb,
                    out_dram, routed, xpool, hpool, xtag, oslices=None):
            oslices = oslices or tslices
            tn_h = tslices[0][1]
            # ---- load activations (transposed [d, token]), slice-major ----
            xt = [xpool.tile([128, T], mm_dt, tag=xtag, name=f"{xtag}{dk}")
                  for dk in range(DB)]
            w1_tiles = {}

            def w1_get(hb):
                if hb not in w1_tiles:
                    t = w1_pool.tile([128, DB, 128], mm_dt, tag="w1")
                    nc.sync.dma_start(
                        out=t,
                        in_=w1_dram[hb].rearrange("(a p) w -> p a w", p=128))
                    w1_tiles[hb] = t
                return w1_tiles[hb]

            for i, (t0, tn) in enumerate(tslices):
                for dk in range(DB):
                    nc.sync.dma_start(
                        out=xt[dk][:, t0:t0 + tn],
                        in_=xT_dram[dk * 128:(dk + 1) * 128, t0:t0 + tn])
                if i == 0:
                    w1_get(0)
                    w1_get(1)

            rs_b = bc_pool.tile([128, T], f32, tag="bc")
            gate_b = None
            if routed:
                gate_b = bc_pool.tile([128, T], f32, tag="bc")

            for (t0, tn) in tslices:
                sl = slice(t0, t0 + tn)
                # ---- 1/rms row:  rs = 1/sqrt(mean(x^2) + eps) ----
                ps_ssq = ps_m.tile([1, tn], f32, tag="psrow")
                for dk in range(DB):
                    sq = sq_pool.tile([128, tn], mm_dt, tag="sqr")
                    nc.scalar.activation(sq, xt[dk][:, sl], AF.Square)
                    nc.tensor.matmul(ps_ssq, ones_col, sq,
                                     start=(dk == 0), stop=(dk == DB - 1))
                r_sqrt = row_pool.tile([1, tn], f32, tag="row")
                nc.scalar.activation(r_sqrt, ps_ssq, AF.Sqrt,
                                     bias=eps_row, scale=1.0 / D)
                r_rs = row_pool.tile([1, tn], f32, tag="row")
                nc.vector.reciprocal(r_rs, r_sqrt)
                bcast(rs_b[:, sl], r_rs, tn)

                if routed:
                    # gate row = affinity = (x @ wre) * rs  (wre norm_w-folded)
                    ps_aff = ps_m.tile([1, tn], f32, tag="psrow")
                    for dk in range(DB):
                        nc.tensor.matmul(ps_aff, wre_sb[:, dk:dk + 1],
                                         xt[dk][:, sl],
                                         start=(dk == 0), stop=(dk == DB - 1))
                    r_gate = row_pool.tile([1, tn], f32, tag="row")
                    nc.vector.tensor_mul(r_gate, ps_aff, r_rs)
                    bcast(gate_b[:, sl], r_gate, tn)

            # ---- mm1 + silu, hb-major: one weight tile closes a full group
            ht = {}
            for si, (t0, tn) in enumerate(tslices):
                for hb in range(HB):
                    w1t = w1_get(hb)
                    sl = slice(t0, t0 + tn)
                    psh = ps_h.tile([128, tn], f32, tag="psh")
                    for dk in range(DB):
                        nc.tensor.matmul(psh, w1t[:, dk, :], xt[dk][:, sl],
                                         start=(dk == 0), stop=(dk == DB - 1))
                    tmp = sq_pool.tile([128, tn], f32, tag="sq")
                    nc.vector.tensor_mul(tmp, psh, rs_b[:, sl])
                    h = hpool.tile([128, tn], mm_dt, tag="h")
                    if silu_decompose:
                        sg = sq_pool.tile([128, tn], f32, tag="sq")
                        nc.scalar.activation(sg, tmp, AF.Sigmoid,
                                             bias=b1_sb[:, hb:hb + 1])
                        nc.vector.scalar_tensor_tensor(
                            h, tmp, b1_sb[:, hb:hb + 1], sg,
                            op0=mybir.AluOpType.add,
                            op1=mybir.AluOpType.mult)
                    else:
                        nc.scalar.activation(h, tmp, AF.Silu,
                                             bias=b1_sb[:, hb:hb + 1])
                    ht[(hb, si)] = h

            # ---- mm2 + combine, db-major ----
            for db in range(DB):
                w2t = w2_pool.tile([128, HB, 128], mm_dt, tag="w2")
                nc.sync.dma_start(
                    out=w2t,
                    in_=w2_dram[db].rearrange("(a p) w -> p a w", p=128))
                for (t0, tn) in (oslices if db >= DB - 2 else tslices):
                    sl = slice(t0, t0 + tn)
                    hi, ho = t0 // tn_h, t0 % tn_h
                    pso = ps_o.tile([128, tn], f32, tag="pso")
                    for hb in range(HB):
                        nc.tensor.matmul(pso, w2t[:, hb, :],
                                         ht[(hb, hi)][:, ho:ho + tn],
                                         start=(hb == 0), stop=(hb == HB - 1))
                    osb = out_pool.tile([128, tn], f32, tag="osb")
                    if routed:
                        nc.vector.scalar_tensor_tensor(
                            osb, pso, b2_sb[:, db:db + 1], gate_b[:, sl],
                            op0=mybir.AluOpType.add,
                            op1=mybir.AluOpType.mult)
                    else:
                        nc.scalar.activation(osb, pso, AF.Identity,
                                             bias=b2_sb[:, db:db + 1])
                    nc.sync.dma_start(
                        out=out_dram[db * 128:(db + 1) * 128, sl], in_=osb)

        # routed job (bigger) first so its weight DMAs start immediately
        nsl = max(1, -(-c // 512))
        base, rem = divmod(c, nsl)
        tsl2, off = [], 0
        for i in range(nsl):
            tn = base + (1 if i < rem else 0)
            tsl2.append((off, tn))
            off += tn
        ffn_job(xgT_d, c, tsl2,
                w1r_d, w2r_d, b1r_sb, b2r_sb, ort_d, True,
                xg_pool, hg_pool, "xg")
        ffn_job(xsT_d, ts, [(0, ts)], w1s_d, w2s_d,
                b1s_sb, b2s_sb, osh_d, False, xs_pool, hs_pool, "xs",
                oslices=[(0, ts // 2), (ts // 2, ts // 2)])

    nc.compile()
    return nc


def _get_nc(key):
    if key not in _CACHE:
        _CACHE[key] = _build(*key)
    return _CACHE[key]


def kernel(x, norm_w, Wr, Ws1, bs1, Ws2, bs2, W1, b1, W2, b2):
    out, _ = _run(x, norm_w, Wr, Ws1, bs1, Ws2, bs2, W1, b1, W2, b2)
    return out


def _run(x, norm_w, Wr, Ws1, bs1, Ws2, bs2, W1, b1, W2, b2,
         trace=False, trace_kwargs=None):
    from concourse.bass_utils import run_bass_kernel_spmd

    x = np.ascontiguousarray(np.asarray(x, dtype=np.float32))
    B, S, _ = x.shape
    xf = x.reshape(NT, D)
    norm_w = np.asarray(norm_w, dtype=np.float32)
    nw64 = norm_w.astype(np.float64)

    # ---- host: routing decisions only (selection, not values) ----
    ms = np.mean(xf.astype(np.float64) ** 2, axis=-1, keepdims=True)
    xn64 = xf.astype(np.float64) / np.sqrt(ms + EPS) * nw64
    aff = xn64 @ np.asarray(Wr, dtype=np.float32).astype(np.float64).T
    part = np.argpartition(-aff, 2, axis=1)[:, :2]  # top-2 set per token
    onehot = np.zeros((NT, 8), dtype=bool)
    onehot[np.arange(NT)[:, None], part] = True
    sel = [np.nonzero(onehot[:, e])[0] for e in range(8)]
    counts = np.array([len(s) for s in sel])
    c_eff = C
    if counts.max() > C:  # input drift beyond planned capacity: rebuild wider
        c_eff = int(-(-int(counts.max()) // 128) * 128)

    # fold norm_w into the first-layer weights (exact when norm_w == 1)
    W1f = (np.asarray(W1, np.float64) * nw64[None, :, None]).astype(np.float32)
    Ws1f = (np.asarray(Ws1, np.float64) * nw64[None, :, None]).astype(np.float32)
    Wrf = (np.asarray(Wr, np.float64) * nw64[None, :]).astype(np.float32)
    Ws2 = np.asarray(Ws2, np.float32)
    W2 = np.asarray(W2, np.float32)
    bs1 = np.asarray(bs1, np.float32)
    bs2 = np.asarray(bs2, np.float32)
    b1 = np.asarray(b1, np.float32)
    b2 = np.asarray(b2, np.float32)

    in_maps = []
    for e in range(8):
        se, q = divmod(e, 4)
        tsl = slice(q * TS, (q + 1) * TS)
        xg = np.zeros((c_eff, D), np.float32)
        xg[:counts[e]] = xf[sel[e]]
        in_maps.append(dict(
            xsT=np.ascontiguousarray(xf[tsl].T),
            xgT=np.ascontiguousarray(xg.T),
            w1s=np.ascontiguousarray(
                Ws1f[se].reshape(D, HB, 128).transpose(1, 0, 2)),
            w2s=np.ascontiguousarray(
                Ws2[se].reshape(H, DB, 128).transpose(1, 0, 2)),
            w1r=np.ascontiguousarray(
                W1f[e].reshape(D, HB, 128).transpose(1, 0, 2)),
            w2r=np.ascontiguousarray(
                W2[e].reshape(H, DB, 128).transpose(1, 0, 2)),
            biases=np.ascontiguousarray(np.concatenate([
                bs1[se, 0].reshape(HB, 128).T,
                bs2[se, 0].reshape(DB, 128).T,
                b1[e, 0].reshape(HB, 128).T,
                b2[e, 0].reshape(DB, 128).T], axis=1)),
            wro=np.ascontiguousarray(np.concatenate([
                Wrf[e].reshape(DB, 128).T,
                np.ones((128, 1), np.float32)], axis=1)),
            onesrow=np.ones((1, 128), np.float32),
        ))

    nc = _get_nc((MM_DT, AUX_DT, TS, c_eff, SILU_DECOMPOSE))
    kw = dict(trace_kwargs or {})
    res = run_bass_kernel_spmd(nc, in_maps, list(range(8)), trace=trace, **kw)

    # ---- host: combine partials (allreduce / all-to-all-combine stand-in) ----
    outf = xf.astype(np.float32).copy()
    for e in range(8):
        q = e % 4
        outf[q * TS:(q + 1) * TS] += res.results[e]["out_sh"].T
        outf[sel[e]] += res.results[e]["out_rt"][:, :counts[e]].T
    return outf.reshape(B, S, D), res
